# revision 11
# baseline (speedup 1.0000x reference)
"""Trainium2 Bass kernel for the DLI loss problem.

Math: in the reference, logits[b,j,k] = a[b,j] + t[b,j+2+k] + fc_b where
a = src_last @ Wh (the 2-step LSTM head) is constant over k. In
loss = LSE_k(logits) - logits[...,0] the a + fc_b terms cancel exactly, so

    loss[b,j] = log(sum_{m=j+2}^{T-1} exp(t[b,m])) - t[b,j+2]
    t[b,m]    = (seg_sum[b,m] @ We) / len[b,m],   We = fc_w[0, H:]

The kernel therefore streams encoder_output (the memory-bound part),
computes ragged segment sums via a PE matmul against an on-device-built
membership matrix, dots with We, and does the tiny LSE-tail epilogue.
Data-parallel over batch: 4 samples per core on 8 cores.

Device layout per sample: partition p holds rows s = 32p + n (n in 0..31),
so every DMA is contiguous per partition. Membership is built from
C[p,(n,u)] = sign(ends_ext[u] + 0.5 - n - 32p) and M = C[:, :, 1:] - C[:, :, :-1]
(values {0,2}; the 1/2 is folded into the host-provided recip = 0.5/len).
"""

import numpy as np
from contextlib import ExitStack

B, S, E, H, T = 32, 4096, 256, 256, 64
NCORES = 8
BPC = B // NCORES      # 4 samples per core
J = T - 2              # 62
NCH = 32               # chunks per sample; s = 32*p + n
U = T + 1              # 65 boundaries (ends_ext = [-1, ends...])
VLEN = NCH * U         # 2080

_cache = {}


def _build(reps=1, enc_mode="split2", drop=""):
    import concourse.bacc as bacc
    import concourse.tile as tile
    import concourse.mybir as mybir

    f32 = mybir.dt.float32
    bf16 = mybir.dt.bfloat16
    f8 = mybir.dt.float8e4
    i32 = mybir.dt.int32
    Alu = mybir.AluOpType
    Act = mybir.ActivationFunctionType
    DR = mybir.MatmulPerfMode.DoubleRow
    fp8 = enc_mode.startswith("fp8")
    edt = f8 if fp8 else bf16

    nc = bacc.Bacc("TRN2", target_bir_lowering=False, debug=False)
    enc = nc.dram_tensor("enc", [BPC, S, E], f32, kind="ExternalInput").ap()
    vth = nc.dram_tensor("vth", [BPC, 3, VLEN], bf16, kind="ExternalInput").ap()
    lhs2 = nc.dram_tensor("lhs2", [128, 128], bf16, kind="ExternalInput").ap()
    recip = nc.dram_tensor("recip", [T, BPC], f32, kind="ExternalInput").ap()
    weh_bc = nc.dram_tensor("weh_bc", [T, E], f32, kind="ExternalInput").ap()
    tri = nc.dram_tensor("tri", [128, J], f32, kind="ExternalInput").ap()
    suf_o = nc.dram_tensor("suf", [J, BPC], f32, kind="ExternalOutput").ap()
    tvals_o = nc.dram_tensor("tvals", [T, BPC], f32, kind="ExternalOutput").ap()

    with tile.TileContext(nc) as tc, ExitStack() as ctx:
        const = ctx.enter_context(tc.tile_pool(name="const", bufs=1))
        encp = ctx.enter_context(tc.tile_pool(name="encp", bufs=2))
        if enc_mode != "gpcast":
            nfb = BPC if fp8 else 2
            encfp = ctx.enter_context(tc.tile_pool(name="encfp", bufs=nfb))
        cp = ctx.enter_context(tc.tile_pool(name="cp", bufs=2))
        mp = ctx.enter_context(tc.tile_pool(name="mp", bufs=2))
        scrp = ctx.enter_context(tc.tile_pool(name="scr", bufs=2))
        smallp = ctx.enter_context(tc.tile_pool(name="small", bufs=2))
        segp = ctx.enter_context(tc.tile_pool(name="seg", bufs=2, space="PSUM"))
        vpsp = ctx.enter_context(tc.tile_pool(name="vps", bufs=2, space="PSUM"))
        vtailp = ctx.enter_context(tc.tile_pool(name="vtail", bufs=1, space="PSUM"))
        psmall = ctx.enter_context(tc.tile_pool(name="psmall", bufs=1, space="PSUM"))

        # constants (matmuls padded to K=128 — K<128 crashes this runtime)
        lhs2_sb = const.tile([128, 128], bf16)
        nc.sync.dma_start(lhs2_sb[:], lhs2[:])
        we_bc = const.tile([T, E], f32)
        nc.sync.dma_start(we_bc[:], weh_bc[:])
        tri_sb = const.tile([128, J], f32)
        nc.sync.dma_start(tri_sb[:], tri[:])
        recip_sb = const.tile([T, BPC], f32)
        nc.sync.dma_start(recip_sb[:], recip[:])
        t_all = const.tile([T, BPC], f32)
        e_all = const.tile([128, BPC], f32)
        nc.vector.memset(e_all[:], 0.0)

        # v rhs tiles (A/B alternating): rows 0..2 per-sample, rows 3..127 zero
        vrhs_ab = [const.tile([128, VLEN], bf16, tag=f"vrhs{i}", name=f"vrhs{i}")
                   for i in range(2)]
        for vt in vrhs_ab:
            nc.vector.memset(vt[:], 0.0)

        for rep in range(reps):
          if fp8:
            # issue all samples' enc loads up front, split across the two
            # HWDGE rings (sync + scalar) to stay within ring capacity
            enc_fs = []
            for b in range(BPC):
                src = enc[b].rearrange("(p n) e -> p n e", p=128)
                ef = encfp.tile([128, NCH, E], f32)
                eng = nc.sync if b % 2 == 0 else nc.scalar
                for h in range(2):
                    eng.dma_start(ef[:, h * 16:(h + 1) * 16, :],
                                  src[:, h * 16:(h + 1) * 16, :])
                enc_fs.append(ef)
          for b in range(BPC):
            it = rep * BPC + b
            src = enc[b].rearrange("(p n) e -> p n e", p=128)
            if "encdma" in drop:
                enc_t = encp.tile([128, NCH, E], bf16)
                nc.vector.memset(enc_t[:], 0.5)
            elif enc_mode == "gpcast":
                # SWDGE DMA casts f32 -> bf16 in flight
                enc_t = encp.tile([128, NCH, E], bf16)
                for h in range(2):
                    nc.gpsimd.dma_start(enc_t[:, h * 16:(h + 1) * 16, :],
                                        src[:, h * 16:(h + 1) * 16, :])
            elif fp8:
                # HWDGE f32 loads (all samples' DMAs issued up front via the
                # 4-deep encfp pool so the DMA queue streams continuously);
                # cast f32 -> fp8e4 split DVE/ACT. Seg matmuls run fp8
                # DoubleRow (2 chunks per matmul, 2 contraction rows/cycle)
                # so the PE stops being the bottleneck. fp8e4 quantization
                # error on the final loss is ~1e-4 rel, well inside 2e-2.
                nd = 6                 # per 16-chunk: DVE n-slices (ACT rest)
                enc_f = enc_fs[b]
                enc_t = encp.tile([128, NCH, E], f8)
                for h in range(2):
                    s0 = h * 16
                    nc.vector.tensor_copy(enc_t[:, s0:s0 + nd, :],
                                          enc_f[:, s0:s0 + nd, :])
                    nc.scalar.activation(enc_t[:, s0 + nd:s0 + 16, :],
                                         enc_f[:, s0 + nd:s0 + 16, :],
                                         Act.Copy)
            elif enc_mode.startswith("split"):
                # HWDGE f32 loads (no Q7 in the loop), cast f32->bf16 on-chip
                # split across DVE / ACT / GPSIMD so no engine bottlenecks.
                nchunk = int(enc_mode[5:])      # chunks per sample
                cw = NCH // nchunk              # n's per chunk
                enc_f = encfp.tile([128, NCH, E], f32)
                for h in range(nchunk):
                    nc.sync.dma_start(enc_f[:, h * cw:(h + 1) * cw, :],
                                      src[:, h * cw:(h + 1) * cw, :])
                enc_t = encp.tile([128, NCH, E], bf16)
                # per chunk: DVE ~3/8, ACT ~3/8, GPSIMD ~2/8 of the n's
                for h in range(nchunk):
                    s0 = h * cw
                    d = max(1, (3 * cw) // 8)
                    a = max(1, (3 * cw) // 8)
                    nc.vector.tensor_copy(enc_t[:, s0:s0 + d, :],
                                          enc_f[:, s0:s0 + d, :])
                    nc.scalar.activation(enc_t[:, s0 + d:s0 + d + a, :],
                                         enc_f[:, s0 + d:s0 + d + a, :],
                                         Act.Copy)
                    nc.gpsimd.tensor_copy(enc_t[:, s0 + d + a:s0 + cw, :],
                                          enc_f[:, s0 + d + a:s0 + cw, :])
            else:
                # HWDGE f32 load + DVE cast to bf16
                enc_f = encfp.tile([128, NCH, E], f32)
                for h in range(2):
                    nc.sync.dma_start(enc_f[:, h * 16:(h + 1) * 16, :],
                                      src[:, h * 16:(h + 1) * 16, :])
                enc_t = encp.tile([128, NCH, E], bf16)
                for h in range(2):
                    nc.vector.tensor_copy(enc_t[:, h * 16:(h + 1) * 16, :],
                                          enc_f[:, h * 16:(h + 1) * 16, :])

            vrhs = vrhs_ab[it % 2]
            M = mp.tile([128, NCH, T], edt)
            if "vgen" in drop:
                nc.vector.memset(M[:], 0.0)
            else:
                nc.sync.dma_start(vrhs[0:3, :], vth[b])
                # v = v_hi + v_lo - 32p via K=128 bf16 matmuls (all terms
                # exact), then C[p,(n,u)] = (v > 0) in {0,1} via DVE compare
                # (no ACT tables involved; ACT stays on Exp the whole kernel).
                C = cp.tile([128, VLEN], bf16)
                for k, (lo, hi) in enumerate(((0, 1024), (1024, 2048), (2048, VLEN))):
                    pool = vpsp if k < 2 else vtailp
                    v_ps = pool.tile([128, hi - lo], f32,
                                     tag="vps" if k < 2 else "vtail")
                    for s0 in range(lo, hi, 512):
                        s1 = min(s0 + 512, hi)
                        nc.tensor.matmul(v_ps[:, s0 - lo:s1 - lo], lhsT=lhs2_sb[:],
                                         rhs=vrhs[:, s0:s1], start=True, stop=True)
                    nc.vector.tensor_scalar(C[:, lo:hi], v_ps[:], 0.0, None,
                                            Alu.is_gt)
                # M[p,(n,t)] = C[p,(n,t+1)] - C[p,(n,t)] in {0, 1}
                C3 = C[:].rearrange("p (n u) -> p n u", u=U)
                nc.vector.tensor_tensor(M[:], C3[:, :, 1:U], C3[:, :, 0:T],
                                        Alu.subtract)

            # seg_sum[t, e] accumulated over the 32 chunks
            seg_ps = segp.tile([T, E], f32)
            nseg = 4 if "seg" in drop else NCH
            if fp8:
                for c in range(nseg // 2):
                    nc.tensor.matmul(
                        seg_ps[:], lhsT=M[:, 2 * c:2 * c + 2, :],
                        rhs=enc_t[:, 2 * c:2 * c + 2, :],
                        start=(c == 0), stop=(c == nseg // 2 - 1),
                        perf_mode=DR,
                    )
            else:
                for n in range(nseg):
                    nc.tensor.matmul(
                        seg_ps[:], lhsT=M[:, n, :], rhs=enc_t[:, n, :],
                        start=(n == 0), stop=(n == nseg - 1),
                    )

            if "epi" in drop:
                nc.vector.tensor_copy(t_all[:, b:b + 1], seg_ps[:, 0:1])
                nc.vector.tensor_copy(e_all[0:T, b:b + 1], seg_ps[:, 0:1])
            else:
                # t_sum[t] = sum_e seg[t,e] * We[e]; t = t_sum / len
                scr = scrp.tile([T, E], f32)
                nc.vector.tensor_tensor(scr[:], seg_ps[:], we_bc[:], Alu.mult)
                tsum = smallp.tile([T, 1], f32, tag="tsum")
                nc.vector.tensor_reduce(tsum[:], scr[:], axis=mybir.AxisListType.X,
                                        op=Alu.add)
                nc.vector.tensor_tensor(t_all[:, b:b + 1], tsum[:],
                                        recip_sb[:, b:b + 1], Alu.mult)
                nc.scalar.activation(e_all[0:T, b:b + 1], t_all[:, b:b + 1],
                                     Act.Exp)

        # suffix sums over exp(t): suf[j,b] = sum_{m>=j+2} e[m,b]
        suf_ps = psmall.tile([J, BPC], f32, tag="ps_small")
        nc.tensor.matmul(suf_ps[:], lhsT=tri_sb[:], rhs=e_all[:],
                         start=True, stop=True)
        suf_sb = const.tile([J, BPC], f32)
        nc.vector.tensor_copy(suf_sb[:], suf_ps[:])
        nc.sync.dma_start(suf_o[:], suf_sb[:])
        nc.sync.dma_start(tvals_o[:], t_all[:])

    nc.compile()
    return nc


def _build_hostm(reps=1, nd=10, v2=False, mv=2):
    """fp8 DoubleRow kernel with HOST-built membership.

    The membership matrix M (one-hot over segments, {0,1} exact in fp8e4)
    is built on the host and DMAed in (1 MiB/core, ~2% extra HBM traffic).
    This removes the whole on-device membership build (v-matmuls, compares,
    subtracts, memsets) from the critical path. All enc loads are issued
    up front on the sync HWDGE ring so the DMA queue streams continuously;
    M rides the scalar ring. Casts f32->fp8 split DVE(nd)/ACT(16-nd) per
    16-chunk. Segment sums run as fp8 DoubleRow matmuls (2 chunks, i.e.
    K=256, per instruction). Epilogue is one fused DVE op per sample:
    t[:,b] = sum_e((seg_ps * recip) * We) via accum_out.
    """
    import concourse.bacc as bacc
    import concourse.tile as tile
    import concourse.mybir as mybir

    f32 = mybir.dt.float32
    f8 = mybir.dt.float8e4
    Alu = mybir.AluOpType
    Act = mybir.ActivationFunctionType
    DR = mybir.MatmulPerfMode.DoubleRow

    nc = bacc.Bacc("TRN2", target_bir_lowering=False, debug=False)
    enc = nc.dram_tensor("enc", [BPC, S, E], f32, kind="ExternalInput").ap()
    m_all = nc.dram_tensor("m_all", [128, BPC * NCH * T], f8,
                           kind="ExternalInput").ap()
    recip = nc.dram_tensor("recip", [T, BPC], f32, kind="ExternalInput").ap()
    weh_bc = nc.dram_tensor("weh_bc", [T, E], f32, kind="ExternalInput").ap()
    if not v2:
        tri = nc.dram_tensor("tri", [128, J], f32, kind="ExternalInput").ap()
        suf_o = nc.dram_tensor("suf", [J, BPC], f32, kind="ExternalOutput").ap()
    tvals_o = nc.dram_tensor("tvals", [T, BPC], f32, kind="ExternalOutput").ap()

    with tile.TileContext(nc) as tc, ExitStack() as ctx:
        const = ctx.enter_context(tc.tile_pool(name="const", bufs=1))
        encfp = ctx.enter_context(tc.tile_pool(name="encfp", bufs=BPC))
        encp = ctx.enter_context(tc.tile_pool(name="encp", bufs=2))
        mp = ctx.enter_context(tc.tile_pool(name="mp", bufs=2))
        scrp = ctx.enter_context(tc.tile_pool(name="scr", bufs=2))
        segp = ctx.enter_context(tc.tile_pool(name="seg", bufs=2, space="PSUM"))
        psmall = ctx.enter_context(tc.tile_pool(name="psmall", bufs=1, space="PSUM"))

        # tiny consts ride the scalar ring so the sync ring is enc-only
        ceng = nc.scalar if v2 else nc.sync
        we_bc = const.tile([T, E], f32)
        ceng.dma_start(we_bc[:], weh_bc[:])
        if not v2:
            tri_sb = const.tile([128, J], f32)
            nc.sync.dma_start(tri_sb[:], tri[:])
        recip_sb = const.tile([T, BPC], f32)
        ceng.dma_start(recip_sb[:], recip[:])
        t_all = const.tile([T, BPC], f32)
        if not v2:
            e_all = const.tile([128, BPC], f32)
            nc.vector.memset(e_all[:], 0.0)

        for rep in range(reps):
            m_sb = mp.tile([128, BPC, NCH, T], f8)
            m_src = m_all[:].rearrange("p (b n t) -> p b n t", b=BPC, n=NCH)
            if not v2 or mv == 3:
                # membership on the scalar ring (concurrent with enc stream)
                nc.scalar.dma_start(m_sb[:], m_src)
            # all enc loads up front, back-to-back on the sync ring;
            # in v2 the membership load is slotted into the same FIFO after
            # sample 0 so nothing competes for HBM concurrently
            enc_fs = []
            for b in range(BPC):
                src = enc[b].rearrange("(p n) e -> p n e", p=128)
                ef = encfp.tile([128, NCH, E], f32)
                # last sample's tail in 1MB chunks to shorten the end chain
                cw = 8 if (v2 and b == BPC - 1) else 16
                for h in range(NCH // cw):
                    nc.sync.dma_start(ef[:, h * cw:(h + 1) * cw, :],
                                      src[:, h * cw:(h + 1) * cw, :])
                enc_fs.append(ef)
                if v2 and mv == 2 and b == 0:
                    nc.sync.dma_start(m_sb[:], m_src)

            for b in range(BPC):
                enc_f = enc_fs[b]
                enc_t = encp.tile([128, NCH, E], f8)
                cw = 8 if (v2 and b == BPC - 1) else 16
                cd = nd if cw == 16 else 5
                for h in range(NCH // cw):
                    s0 = h * cw
                    nc.vector.tensor_copy(enc_t[:, s0:s0 + cd, :],
                                          enc_f[:, s0:s0 + cd, :])
                    nc.scalar.activation(enc_t[:, s0 + cd:s0 + cw, :],
                                         enc_f[:, s0 + cd:s0 + cw, :],
                                         Act.Copy)
                seg_ps = segp.tile([T, E], f32)
                for c in range(NCH // 2):
                    nc.tensor.matmul(
                        seg_ps[:], lhsT=m_sb[:, b, 2 * c:2 * c + 2, :],
                        rhs=enc_t[:, 2 * c:2 * c + 2, :],
                        start=(c == 0), stop=(c == NCH // 2 - 1),
                        perf_mode=DR,
                    )
                # t[:,b] = sum_e((seg * recip) * We)  (fused mult+mult+reduce)
                scr = scrp.tile([T, E], f32)
                nc.vector.scalar_tensor_tensor(
                    scr[:], seg_ps[:], recip_sb[:, b:b + 1], we_bc[:],
                    Alu.mult, Alu.mult, accum_out=t_all[:, b:b + 1])
                if v2:
                    # per-sample result column on the scalar ring; only the
                    # last sample's 256B write remains in the tail
                    nc.scalar.dma_start(tvals_o[:, b:b + 1],
                                        t_all[:, b:b + 1])
                else:
                    nc.scalar.activation(e_all[0:T, b:b + 1],
                                         t_all[:, b:b + 1], Act.Exp)

        if not v2:
            # suffix sums over exp(t): suf[j,b] = sum_{m>=j+2} e[m,b]
            suf_ps = psmall.tile([J, BPC], f32, tag="ps_small")
            nc.tensor.matmul(suf_ps[:], lhsT=tri_sb[:], rhs=e_all[:],
                             start=True, stop=True)
            suf_sb = const.tile([J, BPC], f32)
            nc.vector.tensor_copy(suf_sb[:], suf_ps[:])
            nc.sync.dma_start(suf_o[:], suf_sb[:])
            nc.sync.dma_start(tvals_o[:], t_all[:])

    nc.compile()
    return nc


_LAST_DEDUP = True


def _get_nc(reps=1, enc_mode="gpcast", drop=""):
    key = ("nc", reps, enc_mode, drop,
           _LAST_DEDUP if enc_mode.startswith(("hostf8", "hostbf")) else None)
    if key not in _cache:
        if enc_mode.startswith(("hostf8", "hostbf")):
            bf = enc_mode.startswith("hostbf")
            _cache[key] = _build_hostq(reps, dedup=_LAST_DEDUP, bf=bf)
        elif enc_mode.startswith("hostm"):
            v2 = enc_mode.startswith(("hostm2", "hostm3"))
            mv = 3 if enc_mode.startswith("hostm3") else 2
            tail = enc_mode[6:] if v2 else enc_mode[5:]
            nd = int(tail) if tail else 10
            _cache[key] = _build_hostm(reps, nd, v2, mv)
        else:
            _cache[key] = _build(reps, enc_mode, drop)
    return _cache[key]


def _host_prep(ends_all):
    """Per-sample threshold rows, recips, and the triangular constant."""
    import ml_dtypes
    bf = ml_dtypes.bfloat16
    n_idx = np.arange(NCH, dtype=np.float64)
    vths = np.empty((B, 3, VLEN), bf)
    recips = np.empty((B, T), np.float32)
    for b in range(B):
        ends = ends_all[b].astype(np.float64)
        ends_ext = np.concatenate([[-1.0], ends])            # (65,)
        v = (ends_ext[None, :] + 0.5 - n_idx[:, None]).reshape(-1)  # (VLEN,)
        hi = v.astype(bf)
        lo = (v - hi.astype(np.float64)).astype(bf)
        assert (hi.astype(np.float64) + lo.astype(np.float64) == v).all()
        vths[b, 0] = hi
        vths[b, 1] = lo
        vths[b, 2] = bf(-1.0)
        lens = ends - ends_ext[:T]
        recips[b] = (1.0 / lens).astype(np.float32)
    tri = np.zeros((128, J), np.float32)
    tri[:T] = (np.arange(T)[:, None] >= np.arange(J)[None, :] + 2).astype(np.float32)
    lhs2 = np.zeros((128, 128), bf)
    lhs2[0] = 1.0
    lhs2[1] = 1.0
    lhs2[2] = (32.0 * np.arange(128)).astype(bf)
    assert (lhs2[2].astype(np.float64) == 32.0 * np.arange(128)).all()
    return vths, recips, tri, lhs2


def _host_prep_hostm(ends_all):
    """Host-built one-hot membership in fp8 ({0,1} exact), plus recip/tri."""
    import ml_dtypes
    f8 = ml_dtypes.float8_e4m3
    s_idx = np.arange(S)
    m_cores = np.empty((B, 128, NCH, T), f8)
    recips = np.empty((B, T), np.float32)
    for b in range(B):
        ends = ends_all[b].astype(np.int64)
        seg_id = np.searchsorted(ends, s_idx)          # seg of each row
        m_cores[b] = (seg_id.reshape(128, NCH)[:, :, None]
                      == np.arange(T)[None, None, :]).astype(f8)
        starts = np.concatenate([[0], ends[:-1] + 1])
        recips[b] = (1.0 / (ends - starts + 1)).astype(np.float32)
    tri = np.zeros((128, J), np.float32)
    tri[:T] = (np.arange(T)[:, None] >= np.arange(J)[None, :] + 2).astype(np.float32)
    return m_cores, recips, tri


def _build_hostq(reps=1, dedup=True, bf=False, nd=10, cwl=32):
    """Quantized-upload kernel: enc arrives as fp8 (bf=False) or bf16
    (bf=True) — the host performs the dtype conversion while sharding, so
    the device streams 4.2 MB (fp8) or 8.4 MB (bf16) per core instead of
    16.8 MB of f32. Membership is host-built; when every sample shares the
    same turn boundaries (dedup=True) a single [128, NCH, T] matrix serves
    all samples (256 KB instead of 1 MB).

    fp8 path: the DMA tiles feed the DR matmuls directly (no device cast).
    bf16 path: DVE(nd)/ACT(16-nd) casts bf16 -> fp8 per 16-chunk as before.
    Epilogue per sample: one fused DVE scalar_tensor_tensor with accum_out,
    then the per-sample result column rides the scalar ring out.
    """
    import concourse.bacc as bacc
    import concourse.tile as tile
    import concourse.mybir as mybir

    f32 = mybir.dt.float32
    f8 = mybir.dt.float8e4
    bf16 = mybir.dt.bfloat16
    Alu = mybir.AluOpType
    Act = mybir.ActivationFunctionType
    DR = mybir.MatmulPerfMode.DoubleRow
    edt = bf16 if bf else f8

    nc = bacc.Bacc("TRN2", target_bir_lowering=False, debug=False)
    enc = nc.dram_tensor("enc", [BPC, S, E], edt, kind="ExternalInput").ap()
    # dedup=2: every partition's 32 rows lie in a single segment shared by
    # all samples, so membership collapses to one [128, 2, T] pair matrix
    # (16 KB) reused as lhsT by every DR call.
    mlen = 2 * T if dedup == 2 else (NCH * T if dedup else BPC * NCH * T)
    m_all = nc.dram_tensor("m_all", [128, mlen], f8, kind="ExternalInput").ap()
    recip = nc.dram_tensor("recip", [T, BPC], f32, kind="ExternalInput").ap()
    weh_bc = nc.dram_tensor("weh_bc", [T, E], f32, kind="ExternalInput").ap()
    tvals_o = nc.dram_tensor("tvals", [T, BPC], f32, kind="ExternalOutput").ap()

    with tile.TileContext(nc) as tc, ExitStack() as ctx:
        const = ctx.enter_context(tc.tile_pool(name="const", bufs=1))
        encfp = ctx.enter_context(tc.tile_pool(name="encfp", bufs=BPC))
        if bf:
            encp = ctx.enter_context(tc.tile_pool(name="encp", bufs=2))
        scrp = ctx.enter_context(tc.tile_pool(name="scr", bufs=2))
        segp = ctx.enter_context(tc.tile_pool(name="seg", bufs=2, space="PSUM"))

        # membership leads the sync ring (FIFO => ready before sample 0);
        # the tiny consts ride the scalar ring, needed only by the first
        # epilogue several microseconds later
        m_sb = const.tile([128, mlen], f8)
        nc.sync.dma_start(m_sb[:], m_all[:])
        we_bc = const.tile([T, E], f32)
        nc.scalar.dma_start(we_bc[:], weh_bc[:])
        recip_sb = const.tile([T, BPC], f32)
        nc.scalar.dma_start(recip_sb[:], recip[:])
        if dedup:
            m3 = m_sb[:].rearrange("p (n t) -> p n t", n=NCH)
        else:
            m4 = m_sb[:].rearrange("p (b n t) -> p b n t", b=BPC, n=NCH)
        t_all = const.tile([T, BPC], f32)

        for rep in range(reps):
            enc_fs = []
            for b in range(BPC):
                src = enc[b].rearrange("(p n) e -> p n e", p=128)
                ef = encfp.tile([128, NCH, E], edt)
                # one DMA per sample: 8 KB per-partition runs are needed
                # for DMA line rate (smaller chunks crater to ~2 KB runs
                # and ~300 GB/s)
                cw = cwl
                for h in range(NCH // cw):
                    nc.sync.dma_start(ef[:, h * cw:(h + 1) * cw, :],
                                      src[:, h * cw:(h + 1) * cw, :])
                enc_fs.append(ef)

            for b in range(BPC):
                if bf:
                    enc_f = enc_fs[b]
                    enc_t = encp.tile([128, NCH, E], f8)
                    for h in range(2):
                        s0 = h * 16
                        nc.vector.tensor_copy(enc_t[:, s0:s0 + nd, :],
                                              enc_f[:, s0:s0 + nd, :])
                        nc.scalar.activation(enc_t[:, s0 + nd:s0 + 16, :],
                                             enc_f[:, s0 + nd:s0 + 16, :],
                                             Act.Copy)
                else:
                    enc_t = enc_fs[b]
                seg_ps = segp.tile([T, E], f32)
                for c in range(NCH // 2):
                    lhsT = (m3[:, 2 * c:2 * c + 2, :] if dedup
                            else m4[:, b, 2 * c:2 * c + 2, :])
                    nc.tensor.matmul(
                        seg_ps[:], lhsT=lhsT, rhs=enc_t[:, 2 * c:2 * c + 2, :],
                        start=(c == 0), stop=(c == NCH // 2 - 1),
                        perf_mode=DR,
                    )
                # t[:,b] = sum_e((seg * recip) * We)  (fused mult+mult+reduce)
                scr = scrp.tile([T, E], f32)
                nc.vector.scalar_tensor_tensor(
                    scr[:], seg_ps[:], recip_sb[:, b:b + 1], we_bc[:],
                    Alu.mult, Alu.mult, accum_out=t_all[:, b:b + 1])
                nc.scalar.dma_start(tvals_o[:, b:b + 1], t_all[:, b:b + 1])

    nc.compile()
    return nc


def _host_prep_hostq(ends_all):
    """Host-built one-hot membership ({0,1} exact in fp8) + recips.

    Returns (m_cores [B,128,NCH,T] f8, recips [B,T] f32, dedup flag).
    dedup=True when every sample shares identical turn boundaries (the
    common case here); callers then upload m_cores[0] once per core.
    """
    import ml_dtypes
    f8 = ml_dtypes.float8_e4m3
    dedup = bool((ends_all == ends_all[0:1]).all())
    nb = 1 if dedup else B
    s_idx = np.arange(S)
    m_cores = np.empty((nb, 128, NCH, T), f8)
    recips = np.empty((B, T), np.float32)
    for b in range(B):
        ends = ends_all[b].astype(np.int64)
        if b < nb:
            seg_id = np.searchsorted(ends, s_idx)      # seg of each row
            m_cores[b] = (seg_id.reshape(128, NCH)[:, :, None]
                          == np.arange(T)[None, None, :]).astype(f8)
        starts = np.concatenate([[0], ends[:-1] + 1])
        recips[b] = (1.0 / (ends - starts + 1)).astype(np.float32)
    return m_cores, recips, dedup


MODE = "hostf8"


def build_in_maps(inputs, mode=None):
    global _LAST_DEDUP
    mode = mode or MODE
    enc = np.ascontiguousarray(inputs["encoder_output"], dtype=np.float32)
    ends_all = np.asarray(inputs["his_turn_end_ids"]).astype(np.int64)
    We = np.ascontiguousarray(inputs["fc_w"][0, H:], dtype=np.float32)
    weh = np.ascontiguousarray(np.broadcast_to(We[None, :], (T, E)))
    in_maps = []
    if mode.startswith(("hostf8", "hostbf")):
        import ml_dtypes
        edt = ml_dtypes.bfloat16 if mode.startswith("hostbf") else \
            ml_dtypes.float8_e4m3
        enc_q = enc.astype(edt)
        m_cores, recips, dedup = _host_prep_hostq(ends_all)
        _LAST_DEDUP = dedup
        for c in range(NCORES):
            sl = slice(c * BPC, (c + 1) * BPC)
            if dedup:
                m = np.ascontiguousarray(m_cores[0].reshape(128, NCH * T))
            else:
                m = np.ascontiguousarray(
                    m_cores[sl].transpose(1, 0, 2, 3).reshape(128, -1))
            in_maps.append({
                "enc": enc_q[sl],
                "m_all": m,
                "recip": np.ascontiguousarray(recips[sl].T),
                "weh_bc": weh,
            })
    elif mode.startswith("hostm"):
        m_cores, recips, tri = _host_prep_hostm(ends_all)
        v2 = mode.startswith(("hostm2", "hostm3"))
        for c in range(NCORES):
            sl = slice(c * BPC, (c + 1) * BPC)
            m = np.ascontiguousarray(
                m_cores[sl].transpose(1, 0, 2, 3).reshape(128, BPC * NCH * T))
            im = {
                "enc": enc[sl],
                "m_all": m,
                "recip": np.ascontiguousarray(recips[sl].T),
                "weh_bc": weh,
            }
            if not v2:
                im["tri"] = tri
            in_maps.append(im)
    else:
        vths, recips, tri, lhs2 = _host_prep(ends_all)
        for c in range(NCORES):
            sl = slice(c * BPC, (c + 1) * BPC)
            in_maps.append({
                "enc": enc[sl],
                "vth": np.ascontiguousarray(vths[sl]),
                "recip": np.ascontiguousarray(recips[sl].T),
                "weh_bc": weh,
                "tri": tri,
                "lhs2": lhs2,
            })
    return in_maps


def kernel(**inputs):
    from concourse.bass_utils import run_bass_kernel_spmd

    in_maps = build_in_maps(inputs)
    nc = _get_nc(1, MODE)
    res = run_bass_kernel_spmd(nc, in_maps, list(range(NCORES)))
    return finish(res.results)


def finish(results):
    """Host epilogue: loss from per-core outputs (f64 for precision)."""
    total = 0.0
    for c in range(NCORES):
        tvals = results[c]["tvals"].astype(np.float64)       # (T, BPC)
        if "suf" in results[c]:
            suf = results[c]["suf"].astype(np.float64)       # (J, BPC)
        else:
            e = np.exp(tvals)                                # (T, BPC)
            csum = np.cumsum(e[::-1], axis=0)[::-1]          # suffix sums
            suf = csum[2:, :]                                # (J, BPC)
        total += (np.log(suf) - tvals[2:, :]).sum()
    return np.float32(total / (B * J))


if __name__ == "__main__":
    data = dict(np.load("/root/problem/_inputs.npz"))
    out = kernel(**data)
    print("kernel out:", out)



# revision 15
# speedup vs baseline: 1.0222x; 1.0222x over previous
"""Trainium2 Bass kernel for the DLI loss problem.

Math: in the reference, logits[b,j,k] = a[b,j] + t[b,j+2+k] + fc_b where
a = src_last @ Wh (the 2-step LSTM head) is constant over k. In
loss = LSE_k(logits) - logits[...,0] the a + fc_b terms cancel exactly, so

    loss[b,j] = log(sum_{m=j+2}^{T-1} exp(t[b,m])) - t[b,j+2]
    t[b,m]    = (seg_sum[b,m] @ We) / len[b,m],   We = fc_w[0, H:]

The kernel therefore streams encoder_output (the memory-bound part),
computes ragged segment sums via a PE matmul against an on-device-built
membership matrix, dots with We, and does the tiny LSE-tail epilogue.
Data-parallel over batch: 4 samples per core on 8 cores.

Device layout per sample: partition p holds rows s = 32p + n (n in 0..31),
so every DMA is contiguous per partition. Membership is built from
C[p,(n,u)] = sign(ends_ext[u] + 0.5 - n - 32p) and M = C[:, :, 1:] - C[:, :, :-1]
(values {0,2}; the 1/2 is folded into the host-provided recip = 0.5/len).
"""

import numpy as np
from contextlib import ExitStack

B, S, E, H, T = 32, 4096, 256, 256, 64
NCORES = 8
BPC = B // NCORES      # 4 samples per core
J = T - 2              # 62
NCH = 32               # chunks per sample; s = 32*p + n
U = T + 1              # 65 boundaries (ends_ext = [-1, ends...])
VLEN = NCH * U         # 2080

_cache = {}


def _build(reps=1, enc_mode="split2", drop=""):
    import concourse.bacc as bacc
    import concourse.tile as tile
    import concourse.mybir as mybir

    f32 = mybir.dt.float32
    bf16 = mybir.dt.bfloat16
    f8 = mybir.dt.float8e4
    i32 = mybir.dt.int32
    Alu = mybir.AluOpType
    Act = mybir.ActivationFunctionType
    DR = mybir.MatmulPerfMode.DoubleRow
    fp8 = enc_mode.startswith("fp8")
    edt = f8 if fp8 else bf16

    nc = bacc.Bacc("TRN2", target_bir_lowering=False, debug=False)
    enc = nc.dram_tensor("enc", [BPC, S, E], f32, kind="ExternalInput").ap()
    vth = nc.dram_tensor("vth", [BPC, 3, VLEN], bf16, kind="ExternalInput").ap()
    lhs2 = nc.dram_tensor("lhs2", [128, 128], bf16, kind="ExternalInput").ap()
    recip = nc.dram_tensor("recip", [T, BPC], f32, kind="ExternalInput").ap()
    weh_bc = nc.dram_tensor("weh_bc", [T, E], f32, kind="ExternalInput").ap()
    tri = nc.dram_tensor("tri", [128, J], f32, kind="ExternalInput").ap()
    suf_o = nc.dram_tensor("suf", [J, BPC], f32, kind="ExternalOutput").ap()
    tvals_o = nc.dram_tensor("tvals", [T, BPC], f32, kind="ExternalOutput").ap()

    with tile.TileContext(nc) as tc, ExitStack() as ctx:
        const = ctx.enter_context(tc.tile_pool(name="const", bufs=1))
        encp = ctx.enter_context(tc.tile_pool(name="encp", bufs=2))
        if enc_mode != "gpcast":
            nfb = BPC if fp8 else 2
            encfp = ctx.enter_context(tc.tile_pool(name="encfp", bufs=nfb))
        cp = ctx.enter_context(tc.tile_pool(name="cp", bufs=2))
        mp = ctx.enter_context(tc.tile_pool(name="mp", bufs=2))
        scrp = ctx.enter_context(tc.tile_pool(name="scr", bufs=2))
        smallp = ctx.enter_context(tc.tile_pool(name="small", bufs=2))
        segp = ctx.enter_context(tc.tile_pool(name="seg", bufs=2, space="PSUM"))
        vpsp = ctx.enter_context(tc.tile_pool(name="vps", bufs=2, space="PSUM"))
        vtailp = ctx.enter_context(tc.tile_pool(name="vtail", bufs=1, space="PSUM"))
        psmall = ctx.enter_context(tc.tile_pool(name="psmall", bufs=1, space="PSUM"))

        # constants (matmuls padded to K=128 — K<128 crashes this runtime)
        lhs2_sb = const.tile([128, 128], bf16)
        nc.sync.dma_start(lhs2_sb[:], lhs2[:])
        we_bc = const.tile([T, E], f32)
        nc.sync.dma_start(we_bc[:], weh_bc[:])
        tri_sb = const.tile([128, J], f32)
        nc.sync.dma_start(tri_sb[:], tri[:])
        recip_sb = const.tile([T, BPC], f32)
        nc.sync.dma_start(recip_sb[:], recip[:])
        t_all = const.tile([T, BPC], f32)
        e_all = const.tile([128, BPC], f32)
        nc.vector.memset(e_all[:], 0.0)

        # v rhs tiles (A/B alternating): rows 0..2 per-sample, rows 3..127 zero
        vrhs_ab = [const.tile([128, VLEN], bf16, tag=f"vrhs{i}", name=f"vrhs{i}")
                   for i in range(2)]
        for vt in vrhs_ab:
            nc.vector.memset(vt[:], 0.0)

        for rep in range(reps):
          if fp8:
            # issue all samples' enc loads up front, split across the two
            # HWDGE rings (sync + scalar) to stay within ring capacity
            enc_fs = []
            for b in range(BPC):
                src = enc[b].rearrange("(p n) e -> p n e", p=128)
                ef = encfp.tile([128, NCH, E], f32)
                eng = nc.sync if b % 2 == 0 else nc.scalar
                for h in range(2):
                    eng.dma_start(ef[:, h * 16:(h + 1) * 16, :],
                                  src[:, h * 16:(h + 1) * 16, :])
                enc_fs.append(ef)
          for b in range(BPC):
            it = rep * BPC + b
            src = enc[b].rearrange("(p n) e -> p n e", p=128)
            if "encdma" in drop:
                enc_t = encp.tile([128, NCH, E], bf16)
                nc.vector.memset(enc_t[:], 0.5)
            elif enc_mode == "gpcast":
                # SWDGE DMA casts f32 -> bf16 in flight
                enc_t = encp.tile([128, NCH, E], bf16)
                for h in range(2):
                    nc.gpsimd.dma_start(enc_t[:, h * 16:(h + 1) * 16, :],
                                        src[:, h * 16:(h + 1) * 16, :])
            elif fp8:
                # HWDGE f32 loads (all samples' DMAs issued up front via the
                # 4-deep encfp pool so the DMA queue streams continuously);
                # cast f32 -> fp8e4 split DVE/ACT. Seg matmuls run fp8
                # DoubleRow (2 chunks per matmul, 2 contraction rows/cycle)
                # so the PE stops being the bottleneck. fp8e4 quantization
                # error on the final loss is ~1e-4 rel, well inside 2e-2.
                nd = 6                 # per 16-chunk: DVE n-slices (ACT rest)
                enc_f = enc_fs[b]
                enc_t = encp.tile([128, NCH, E], f8)
                for h in range(2):
                    s0 = h * 16
                    nc.vector.tensor_copy(enc_t[:, s0:s0 + nd, :],
                                          enc_f[:, s0:s0 + nd, :])
                    nc.scalar.activation(enc_t[:, s0 + nd:s0 + 16, :],
                                         enc_f[:, s0 + nd:s0 + 16, :],
                                         Act.Copy)
            elif enc_mode.startswith("split"):
                # HWDGE f32 loads (no Q7 in the loop), cast f32->bf16 on-chip
                # split across DVE / ACT / GPSIMD so no engine bottlenecks.
                nchunk = int(enc_mode[5:])      # chunks per sample
                cw = NCH // nchunk              # n's per chunk
                enc_f = encfp.tile([128, NCH, E], f32)
                for h in range(nchunk):
                    nc.sync.dma_start(enc_f[:, h * cw:(h + 1) * cw, :],
                                      src[:, h * cw:(h + 1) * cw, :])
                enc_t = encp.tile([128, NCH, E], bf16)
                # per chunk: DVE ~3/8, ACT ~3/8, GPSIMD ~2/8 of the n's
                for h in range(nchunk):
                    s0 = h * cw
                    d = max(1, (3 * cw) // 8)
                    a = max(1, (3 * cw) // 8)
                    nc.vector.tensor_copy(enc_t[:, s0:s0 + d, :],
                                          enc_f[:, s0:s0 + d, :])
                    nc.scalar.activation(enc_t[:, s0 + d:s0 + d + a, :],
                                         enc_f[:, s0 + d:s0 + d + a, :],
                                         Act.Copy)
                    nc.gpsimd.tensor_copy(enc_t[:, s0 + d + a:s0 + cw, :],
                                          enc_f[:, s0 + d + a:s0 + cw, :])
            else:
                # HWDGE f32 load + DVE cast to bf16
                enc_f = encfp.tile([128, NCH, E], f32)
                for h in range(2):
                    nc.sync.dma_start(enc_f[:, h * 16:(h + 1) * 16, :],
                                      src[:, h * 16:(h + 1) * 16, :])
                enc_t = encp.tile([128, NCH, E], bf16)
                for h in range(2):
                    nc.vector.tensor_copy(enc_t[:, h * 16:(h + 1) * 16, :],
                                          enc_f[:, h * 16:(h + 1) * 16, :])

            vrhs = vrhs_ab[it % 2]
            M = mp.tile([128, NCH, T], edt)
            if "vgen" in drop:
                nc.vector.memset(M[:], 0.0)
            else:
                nc.sync.dma_start(vrhs[0:3, :], vth[b])
                # v = v_hi + v_lo - 32p via K=128 bf16 matmuls (all terms
                # exact), then C[p,(n,u)] = (v > 0) in {0,1} via DVE compare
                # (no ACT tables involved; ACT stays on Exp the whole kernel).
                C = cp.tile([128, VLEN], bf16)
                for k, (lo, hi) in enumerate(((0, 1024), (1024, 2048), (2048, VLEN))):
                    pool = vpsp if k < 2 else vtailp
                    v_ps = pool.tile([128, hi - lo], f32,
                                     tag="vps" if k < 2 else "vtail")
                    for s0 in range(lo, hi, 512):
                        s1 = min(s0 + 512, hi)
                        nc.tensor.matmul(v_ps[:, s0 - lo:s1 - lo], lhsT=lhs2_sb[:],
                                         rhs=vrhs[:, s0:s1], start=True, stop=True)
                    nc.vector.tensor_scalar(C[:, lo:hi], v_ps[:], 0.0, None,
                                            Alu.is_gt)
                # M[p,(n,t)] = C[p,(n,t+1)] - C[p,(n,t)] in {0, 1}
                C3 = C[:].rearrange("p (n u) -> p n u", u=U)
                nc.vector.tensor_tensor(M[:], C3[:, :, 1:U], C3[:, :, 0:T],
                                        Alu.subtract)

            # seg_sum[t, e] accumulated over the 32 chunks
            seg_ps = segp.tile([T, E], f32)
            nseg = 4 if "seg" in drop else NCH
            if fp8:
                for c in range(nseg // 2):
                    nc.tensor.matmul(
                        seg_ps[:], lhsT=M[:, 2 * c:2 * c + 2, :],
                        rhs=enc_t[:, 2 * c:2 * c + 2, :],
                        start=(c == 0), stop=(c == nseg // 2 - 1),
                        perf_mode=DR,
                    )
            else:
                for n in range(nseg):
                    nc.tensor.matmul(
                        seg_ps[:], lhsT=M[:, n, :], rhs=enc_t[:, n, :],
                        start=(n == 0), stop=(n == nseg - 1),
                    )

            if "epi" in drop:
                nc.vector.tensor_copy(t_all[:, b:b + 1], seg_ps[:, 0:1])
                nc.vector.tensor_copy(e_all[0:T, b:b + 1], seg_ps[:, 0:1])
            else:
                # t_sum[t] = sum_e seg[t,e] * We[e]; t = t_sum / len
                scr = scrp.tile([T, E], f32)
                nc.vector.tensor_tensor(scr[:], seg_ps[:], we_bc[:], Alu.mult)
                tsum = smallp.tile([T, 1], f32, tag="tsum")
                nc.vector.tensor_reduce(tsum[:], scr[:], axis=mybir.AxisListType.X,
                                        op=Alu.add)
                nc.vector.tensor_tensor(t_all[:, b:b + 1], tsum[:],
                                        recip_sb[:, b:b + 1], Alu.mult)
                nc.scalar.activation(e_all[0:T, b:b + 1], t_all[:, b:b + 1],
                                     Act.Exp)

        # suffix sums over exp(t): suf[j,b] = sum_{m>=j+2} e[m,b]
        suf_ps = psmall.tile([J, BPC], f32, tag="ps_small")
        nc.tensor.matmul(suf_ps[:], lhsT=tri_sb[:], rhs=e_all[:],
                         start=True, stop=True)
        suf_sb = const.tile([J, BPC], f32)
        nc.vector.tensor_copy(suf_sb[:], suf_ps[:])
        nc.sync.dma_start(suf_o[:], suf_sb[:])
        nc.sync.dma_start(tvals_o[:], t_all[:])

    nc.compile()
    return nc


def _build_hostm(reps=1, nd=10, v2=False, mv=2):
    """fp8 DoubleRow kernel with HOST-built membership.

    The membership matrix M (one-hot over segments, {0,1} exact in fp8e4)
    is built on the host and DMAed in (1 MiB/core, ~2% extra HBM traffic).
    This removes the whole on-device membership build (v-matmuls, compares,
    subtracts, memsets) from the critical path. All enc loads are issued
    up front on the sync HWDGE ring so the DMA queue streams continuously;
    M rides the scalar ring. Casts f32->fp8 split DVE(nd)/ACT(16-nd) per
    16-chunk. Segment sums run as fp8 DoubleRow matmuls (2 chunks, i.e.
    K=256, per instruction). Epilogue is one fused DVE op per sample:
    t[:,b] = sum_e((seg_ps * recip) * We) via accum_out.
    """
    import concourse.bacc as bacc
    import concourse.tile as tile
    import concourse.mybir as mybir

    f32 = mybir.dt.float32
    f8 = mybir.dt.float8e4
    Alu = mybir.AluOpType
    Act = mybir.ActivationFunctionType
    DR = mybir.MatmulPerfMode.DoubleRow

    nc = bacc.Bacc("TRN2", target_bir_lowering=False, debug=False)
    enc = nc.dram_tensor("enc", [BPC, S, E], f32, kind="ExternalInput").ap()
    m_all = nc.dram_tensor("m_all", [128, BPC * NCH * T], f8,
                           kind="ExternalInput").ap()
    recip = nc.dram_tensor("recip", [T, BPC], f32, kind="ExternalInput").ap()
    weh_bc = nc.dram_tensor("weh_bc", [T, E], f32, kind="ExternalInput").ap()
    if not v2:
        tri = nc.dram_tensor("tri", [128, J], f32, kind="ExternalInput").ap()
        suf_o = nc.dram_tensor("suf", [J, BPC], f32, kind="ExternalOutput").ap()
    tvals_o = nc.dram_tensor("tvals", [T, BPC], f32, kind="ExternalOutput").ap()

    with tile.TileContext(nc) as tc, ExitStack() as ctx:
        const = ctx.enter_context(tc.tile_pool(name="const", bufs=1))
        encfp = ctx.enter_context(tc.tile_pool(name="encfp", bufs=BPC))
        encp = ctx.enter_context(tc.tile_pool(name="encp", bufs=2))
        mp = ctx.enter_context(tc.tile_pool(name="mp", bufs=2))
        scrp = ctx.enter_context(tc.tile_pool(name="scr", bufs=2))
        segp = ctx.enter_context(tc.tile_pool(name="seg", bufs=2, space="PSUM"))
        psmall = ctx.enter_context(tc.tile_pool(name="psmall", bufs=1, space="PSUM"))

        # tiny consts ride the scalar ring so the sync ring is enc-only
        ceng = nc.scalar if v2 else nc.sync
        we_bc = const.tile([T, E], f32)
        ceng.dma_start(we_bc[:], weh_bc[:])
        if not v2:
            tri_sb = const.tile([128, J], f32)
            nc.sync.dma_start(tri_sb[:], tri[:])
        recip_sb = const.tile([T, BPC], f32)
        ceng.dma_start(recip_sb[:], recip[:])
        t_all = const.tile([T, BPC], f32)
        if not v2:
            e_all = const.tile([128, BPC], f32)
            nc.vector.memset(e_all[:], 0.0)

        for rep in range(reps):
            m_sb = mp.tile([128, BPC, NCH, T], f8)
            m_src = m_all[:].rearrange("p (b n t) -> p b n t", b=BPC, n=NCH)
            if not v2 or mv == 3:
                # membership on the scalar ring (concurrent with enc stream)
                nc.scalar.dma_start(m_sb[:], m_src)
            # all enc loads up front, back-to-back on the sync ring;
            # in v2 the membership load is slotted into the same FIFO after
            # sample 0 so nothing competes for HBM concurrently
            enc_fs = []
            for b in range(BPC):
                src = enc[b].rearrange("(p n) e -> p n e", p=128)
                ef = encfp.tile([128, NCH, E], f32)
                # last sample's tail in 1MB chunks to shorten the end chain
                cw = 8 if (v2 and b == BPC - 1) else 16
                for h in range(NCH // cw):
                    nc.sync.dma_start(ef[:, h * cw:(h + 1) * cw, :],
                                      src[:, h * cw:(h + 1) * cw, :])
                enc_fs.append(ef)
                if v2 and mv == 2 and b == 0:
                    nc.sync.dma_start(m_sb[:], m_src)

            for b in range(BPC):
                enc_f = enc_fs[b]
                enc_t = encp.tile([128, NCH, E], f8)
                cw = 8 if (v2 and b == BPC - 1) else 16
                cd = nd if cw == 16 else 5
                for h in range(NCH // cw):
                    s0 = h * cw
                    nc.vector.tensor_copy(enc_t[:, s0:s0 + cd, :],
                                          enc_f[:, s0:s0 + cd, :])
                    nc.scalar.activation(enc_t[:, s0 + cd:s0 + cw, :],
                                         enc_f[:, s0 + cd:s0 + cw, :],
                                         Act.Copy)
                seg_ps = segp.tile([T, E], f32)
                for c in range(NCH // 2):
                    nc.tensor.matmul(
                        seg_ps[:], lhsT=m_sb[:, b, 2 * c:2 * c + 2, :],
                        rhs=enc_t[:, 2 * c:2 * c + 2, :],
                        start=(c == 0), stop=(c == NCH // 2 - 1),
                        perf_mode=DR,
                    )
                # t[:,b] = sum_e((seg * recip) * We)  (fused mult+mult+reduce)
                scr = scrp.tile([T, E], f32)
                nc.vector.scalar_tensor_tensor(
                    scr[:], seg_ps[:], recip_sb[:, b:b + 1], we_bc[:],
                    Alu.mult, Alu.mult, accum_out=t_all[:, b:b + 1])
                if v2:
                    # per-sample result column on the scalar ring; only the
                    # last sample's 256B write remains in the tail
                    nc.scalar.dma_start(tvals_o[:, b:b + 1],
                                        t_all[:, b:b + 1])
                else:
                    nc.scalar.activation(e_all[0:T, b:b + 1],
                                         t_all[:, b:b + 1], Act.Exp)

        if not v2:
            # suffix sums over exp(t): suf[j,b] = sum_{m>=j+2} e[m,b]
            suf_ps = psmall.tile([J, BPC], f32, tag="ps_small")
            nc.tensor.matmul(suf_ps[:], lhsT=tri_sb[:], rhs=e_all[:],
                             start=True, stop=True)
            suf_sb = const.tile([J, BPC], f32)
            nc.vector.tensor_copy(suf_sb[:], suf_ps[:])
            nc.sync.dma_start(suf_o[:], suf_sb[:])
            nc.sync.dma_start(tvals_o[:], t_all[:])

    nc.compile()
    return nc


_LAST_DEDUP = True


def _get_nc(reps=1, enc_mode="gpcast", drop=""):
    key = ("nc", reps, enc_mode, drop,
           _LAST_DEDUP if enc_mode.startswith(("hostf8", "hostbf")) else None)
    if key not in _cache:
        if enc_mode.startswith(("hostf8", "hostbf")):
            bf = enc_mode.startswith("hostbf")
            _cache[key] = _build_hostq(reps, dedup=_LAST_DEDUP, bf=bf)
        elif enc_mode.startswith("hostm"):
            v2 = enc_mode.startswith(("hostm2", "hostm3"))
            mv = 3 if enc_mode.startswith("hostm3") else 2
            tail = enc_mode[6:] if v2 else enc_mode[5:]
            nd = int(tail) if tail else 10
            _cache[key] = _build_hostm(reps, nd, v2, mv)
        else:
            _cache[key] = _build(reps, enc_mode, drop)
    return _cache[key]


def _host_prep(ends_all):
    """Per-sample threshold rows, recips, and the triangular constant."""
    import ml_dtypes
    bf = ml_dtypes.bfloat16
    n_idx = np.arange(NCH, dtype=np.float64)
    vths = np.empty((B, 3, VLEN), bf)
    recips = np.empty((B, T), np.float32)
    for b in range(B):
        ends = ends_all[b].astype(np.float64)
        ends_ext = np.concatenate([[-1.0], ends])            # (65,)
        v = (ends_ext[None, :] + 0.5 - n_idx[:, None]).reshape(-1)  # (VLEN,)
        hi = v.astype(bf)
        lo = (v - hi.astype(np.float64)).astype(bf)
        assert (hi.astype(np.float64) + lo.astype(np.float64) == v).all()
        vths[b, 0] = hi
        vths[b, 1] = lo
        vths[b, 2] = bf(-1.0)
        lens = ends - ends_ext[:T]
        recips[b] = (1.0 / lens).astype(np.float32)
    tri = np.zeros((128, J), np.float32)
    tri[:T] = (np.arange(T)[:, None] >= np.arange(J)[None, :] + 2).astype(np.float32)
    lhs2 = np.zeros((128, 128), bf)
    lhs2[0] = 1.0
    lhs2[1] = 1.0
    lhs2[2] = (32.0 * np.arange(128)).astype(bf)
    assert (lhs2[2].astype(np.float64) == 32.0 * np.arange(128)).all()
    return vths, recips, tri, lhs2


def _host_prep_hostm(ends_all):
    """Host-built one-hot membership in fp8 ({0,1} exact), plus recip/tri."""
    import ml_dtypes
    f8 = ml_dtypes.float8_e4m3
    s_idx = np.arange(S)
    m_cores = np.empty((B, 128, NCH, T), f8)
    recips = np.empty((B, T), np.float32)
    for b in range(B):
        ends = ends_all[b].astype(np.int64)
        seg_id = np.searchsorted(ends, s_idx)          # seg of each row
        m_cores[b] = (seg_id.reshape(128, NCH)[:, :, None]
                      == np.arange(T)[None, None, :]).astype(f8)
        starts = np.concatenate([[0], ends[:-1] + 1])
        recips[b] = (1.0 / (ends - starts + 1)).astype(np.float32)
    tri = np.zeros((128, J), np.float32)
    tri[:T] = (np.arange(T)[:, None] >= np.arange(J)[None, :] + 2).astype(np.float32)
    return m_cores, recips, tri


def _build_hostq(reps=1, dedup=True, bf=False, nd=10, cwl=32):
    """Quantized-upload kernel: enc arrives as fp8 (bf=False) or bf16
    (bf=True) — the host performs the dtype conversion while sharding, so
    the device streams 4.2 MB (fp8) or 8.4 MB (bf16) per core instead of
    16.8 MB of f32. Membership is host-built; when every sample shares the
    same turn boundaries (dedup=True) a single [128, NCH, T] matrix serves
    all samples (256 KB instead of 1 MB).

    fp8 path: the DMA tiles feed the DR matmuls directly (no device cast).
    bf16 path: DVE(nd)/ACT(16-nd) casts bf16 -> fp8 per 16-chunk as before.
    Epilogue per sample: one fused DVE scalar_tensor_tensor with accum_out,
    then the per-sample result column rides the scalar ring out.
    """
    import concourse.bacc as bacc
    import concourse.tile as tile
    import concourse.mybir as mybir

    f32 = mybir.dt.float32
    f8 = mybir.dt.float8e4
    bf16 = mybir.dt.bfloat16
    Alu = mybir.AluOpType
    Act = mybir.ActivationFunctionType
    DR = mybir.MatmulPerfMode.DoubleRow
    edt = bf16 if bf else f8

    nc = bacc.Bacc("TRN2", target_bir_lowering=False, debug=False)
    enc = nc.dram_tensor("enc", [BPC, S, E], edt, kind="ExternalInput").ap()
    # dedup=2: every partition's 32 rows lie in a single segment shared by
    # all samples, so membership collapses to one [128, 2, T] pair matrix
    # (16 KB) reused as lhsT by every DR call.
    mlen = 2 * T if dedup == 2 else (NCH * T if dedup else BPC * NCH * T)
    m_all = nc.dram_tensor("m_all", [128, mlen], f8, kind="ExternalInput").ap()
    recip = nc.dram_tensor("recip", [T, BPC], f32, kind="ExternalInput").ap()
    weh_bc = nc.dram_tensor("weh_bc", [T, E], f32, kind="ExternalInput").ap()
    tvals_o = nc.dram_tensor("tvals", [T, BPC], f32, kind="ExternalOutput").ap()

    with tile.TileContext(nc) as tc, ExitStack() as ctx:
        const = ctx.enter_context(tc.tile_pool(name="const", bufs=1))
        encfp = ctx.enter_context(tc.tile_pool(name="encfp", bufs=BPC))
        if bf:
            encp = ctx.enter_context(tc.tile_pool(name="encp", bufs=2))
        scrp = ctx.enter_context(tc.tile_pool(name="scr", bufs=2))
        segp = ctx.enter_context(tc.tile_pool(name="seg", bufs=2, space="PSUM"))

        # membership leads the sync ring (FIFO => ready before sample 0);
        # the tiny consts ride the scalar ring, needed only by the first
        # epilogue several microseconds later
        m_sb = const.tile([128, mlen], f8)
        nc.sync.dma_start(m_sb[:], m_all[:])
        we_bc = const.tile([T, E], f32)
        nc.scalar.dma_start(we_bc[:], weh_bc[:])
        recip_sb = const.tile([T, BPC], f32)
        nc.scalar.dma_start(recip_sb[:], recip[:])
        if dedup == 2:
            m2 = m_sb[:].rearrange("p (j t) -> p j t", j=2)
        elif dedup:
            m3 = m_sb[:].rearrange("p (n t) -> p n t", n=NCH)
        else:
            m4 = m_sb[:].rearrange("p (b n t) -> p b n t", b=BPC, n=NCH)
        t_all = const.tile([T, BPC], f32)

        for rep in range(reps):
            enc_fs = []
            for b in range(BPC):
                src = enc[b].rearrange("(p n) e -> p n e", p=128)
                ef = encfp.tile([128, NCH, E], edt)
                # one DMA per sample: 8 KB per-partition runs are needed
                # for DMA line rate (smaller chunks crater to ~2 KB runs
                # and ~300 GB/s)
                cw = cwl
                for h in range(NCH // cw):
                    nc.sync.dma_start(ef[:, h * cw:(h + 1) * cw, :],
                                      src[:, h * cw:(h + 1) * cw, :])
                enc_fs.append(ef)

            for b in range(BPC):
                if bf:
                    enc_f = enc_fs[b]
                    enc_t = encp.tile([128, NCH, E], f8)
                    for h in range(2):
                        s0 = h * 16
                        nc.vector.tensor_copy(enc_t[:, s0:s0 + nd, :],
                                              enc_f[:, s0:s0 + nd, :])
                        nc.scalar.activation(enc_t[:, s0 + nd:s0 + 16, :],
                                             enc_f[:, s0 + nd:s0 + 16, :],
                                             Act.Copy)
                else:
                    enc_t = enc_fs[b]
                seg_ps = segp.tile([T, E], f32)
                for c in range(NCH // 2):
                    lhsT = (m2[:, :, :] if dedup == 2
                            else m3[:, 2 * c:2 * c + 2, :] if dedup
                            else m4[:, b, 2 * c:2 * c + 2, :])
                    nc.tensor.matmul(
                        seg_ps[:], lhsT=lhsT, rhs=enc_t[:, 2 * c:2 * c + 2, :],
                        start=(c == 0), stop=(c == NCH // 2 - 1),
                        perf_mode=DR,
                    )
                # t[:,b] = sum_e((seg * recip) * We)  (fused mult+mult+reduce)
                scr = scrp.tile([T, E], f32)
                nc.vector.scalar_tensor_tensor(
                    scr[:], seg_ps[:], recip_sb[:, b:b + 1], we_bc[:],
                    Alu.mult, Alu.mult, accum_out=t_all[:, b:b + 1])
                nc.scalar.dma_start(tvals_o[:, b:b + 1], t_all[:, b:b + 1])

    nc.compile()
    return nc


def _host_prep_hostq(ends_all):
    """Host-built one-hot membership ({0,1} exact in fp8) + recips.

    Returns (m_cores [B,128,NCH,T] f8, recips [B,T] f32, dedup flag).
    dedup=True when every sample shares identical turn boundaries (the
    common case here); callers then upload m_cores[0] once per core.
    """
    import ml_dtypes
    f8 = ml_dtypes.float8_e4m3
    dedup = int((ends_all == ends_all[0:1]).all())
    s_idx = np.arange(S)
    recips = np.empty((B, T), np.float32)
    for b in range(B):
        ends = ends_all[b].astype(np.int64)
        starts = np.concatenate([[0], ends[:-1] + 1])
        recips[b] = (1.0 / (ends - starts + 1)).astype(np.float32)
    seg_id = np.searchsorted(ends_all[0].astype(np.int64), s_idx)
    if dedup and (seg_id.reshape(128, NCH)[:, 0]
                  == seg_id.reshape(128, NCH)[:, -1]).all():
        # every partition's 32 rows lie in one segment -> pair matrix
        mp = (seg_id.reshape(128, NCH)[:, 0][:, None]
              == np.arange(T)[None, :]).astype(f8)       # [128, T]
        m_cores = np.ascontiguousarray(
            np.repeat(mp[:, None, :], 2, axis=1))        # [128, 2, T]
        return m_cores, recips, 2
    nb = 1 if dedup else B
    m_cores = np.empty((nb, 128, NCH, T), f8)
    for b in range(nb):
        sid = np.searchsorted(ends_all[b].astype(np.int64), s_idx)
        m_cores[b] = (sid.reshape(128, NCH)[:, :, None]
                      == np.arange(T)[None, None, :]).astype(f8)
    return m_cores, recips, dedup


MODE = "hostf8"


def build_in_maps(inputs, mode=None):
    global _LAST_DEDUP
    mode = mode or MODE
    enc = np.ascontiguousarray(inputs["encoder_output"], dtype=np.float32)
    ends_all = np.asarray(inputs["his_turn_end_ids"]).astype(np.int64)
    We = np.ascontiguousarray(inputs["fc_w"][0, H:], dtype=np.float32)
    weh = np.ascontiguousarray(np.broadcast_to(We[None, :], (T, E)))
    in_maps = []
    if mode.startswith(("hostf8", "hostbf")):
        import ml_dtypes
        edt = ml_dtypes.bfloat16 if mode.startswith("hostbf") else \
            ml_dtypes.float8_e4m3
        enc_q = enc.astype(edt)
        m_cores, recips, dedup = _host_prep_hostq(ends_all)
        _LAST_DEDUP = dedup
        for c in range(NCORES):
            sl = slice(c * BPC, (c + 1) * BPC)
            if dedup == 2:
                m = np.ascontiguousarray(m_cores.reshape(128, 2 * T))
            elif dedup:
                m = np.ascontiguousarray(m_cores[0].reshape(128, NCH * T))
            else:
                m = np.ascontiguousarray(
                    m_cores[sl].transpose(1, 0, 2, 3).reshape(128, -1))
            in_maps.append({
                "enc": enc_q[sl],
                "m_all": m,
                "recip": np.ascontiguousarray(recips[sl].T),
                "weh_bc": weh,
            })
    elif mode.startswith("hostm"):
        m_cores, recips, tri = _host_prep_hostm(ends_all)
        v2 = mode.startswith(("hostm2", "hostm3"))
        for c in range(NCORES):
            sl = slice(c * BPC, (c + 1) * BPC)
            m = np.ascontiguousarray(
                m_cores[sl].transpose(1, 0, 2, 3).reshape(128, BPC * NCH * T))
            im = {
                "enc": enc[sl],
                "m_all": m,
                "recip": np.ascontiguousarray(recips[sl].T),
                "weh_bc": weh,
            }
            if not v2:
                im["tri"] = tri
            in_maps.append(im)
    else:
        vths, recips, tri, lhs2 = _host_prep(ends_all)
        for c in range(NCORES):
            sl = slice(c * BPC, (c + 1) * BPC)
            in_maps.append({
                "enc": enc[sl],
                "vth": np.ascontiguousarray(vths[sl]),
                "recip": np.ascontiguousarray(recips[sl].T),
                "weh_bc": weh,
                "tri": tri,
                "lhs2": lhs2,
            })
    return in_maps


def kernel(**inputs):
    from concourse.bass_utils import run_bass_kernel_spmd

    in_maps = build_in_maps(inputs)
    nc = _get_nc(1, MODE)
    res = run_bass_kernel_spmd(nc, in_maps, list(range(NCORES)))
    return finish(res.results)


def finish(results):
    """Host epilogue: loss from per-core outputs (f64 for precision)."""
    total = 0.0
    for c in range(NCORES):
        tvals = results[c]["tvals"].astype(np.float64)       # (T, BPC)
        if "suf" in results[c]:
            suf = results[c]["suf"].astype(np.float64)       # (J, BPC)
        else:
            e = np.exp(tvals)                                # (T, BPC)
            csum = np.cumsum(e[::-1], axis=0)[::-1]          # suffix sums
            suf = csum[2:, :]                                # (J, BPC)
        total += (np.log(suf) - tvals[2:, :]).sum()
    return np.float32(total / (B * J))


if __name__ == "__main__":
    data = dict(np.load("/root/problem/_inputs.npz"))
    out = kernel(**data)
    print("kernel out:", out)

